# revision 2
# baseline (speedup 1.0000x reference)
"""Trainium2 Bass kernel for nn_DecoderTreeNN (gather + segment_sum over trees).

Computes, for two embedding tables C_hop / C_hop1:
    out[t, seg, :] = sum_{i : tree_ids[i] == seg} C_t[token_ids[i], :]
returning [2, 32, 512, 128] f32.

Strategy (8 NeuronCores, SPMD):
  - 16384 segments -> 128 "windows" of 128 consecutive segments. Core c owns
    windows [16c, 16c+16); since tree_ids is sorted, each window's tokens are
    a contiguous slice of the token stream. Host pads every window to a fixed
    16384 token slots (max real occupancy of this distribution ~15.9k); pad
    tokens use id 0, whose embedding row is all-zero (padding_idx), so they
    contribute nothing.
  - Host concatenates the two tables into one [32000, 256] f32 table, so one
    gathered row (1024 B) serves both outputs.
  - On device, per 4096-token chunk one gpsimd.dma_gather pulls the rows into
    SBUF as [128, 32, 256] (token k = j*128 + p). Per 128-token tile j, the
    DVE builds a selection matrix S[p, s] = (tree_rel[p] == s) by comparing a
    per-partition scalar against an iota row; the PE accumulates
    S^T @ G  ->  PSUM[128 segs, 256] across the window's 128 tiles.
  - PSUM is copied to SBUF and DMA'd to a per-core [16, 128, 256] output;
    the host reassembles the full [2, 32, 512, 128].
"""

from contextlib import ExitStack

import numpy as np

import concourse.bacc as bacc
import concourse.bass as bass
import concourse.mybir as mybir
import concourse.tile as tile
from concourse.bass_utils import run_bass_kernel_spmd
from concourse.library_config import mlp

P = 128
V = 32000
D = 128              # embedding dim per table
DD = 2 * D           # concatenated row width
N_CORES = 8
NSEG = 16384
SEGS_PW = 128        # segments per window
WG = NSEG // SEGS_PW             # 128 global windows
W = WG // N_CORES                # 16 windows per core
CAP = 16384                      # padded tokens per window
CHUNK = 4096                     # tokens per dma_gather
NCH = CAP // CHUNK               # 4 chunks per window
NJ = CHUNK // P                  # 32 token tiles per chunk
NQ = W * NCH                     # 64 chunks per core

_compiled = None


def _build_program():
    nc = bacc.Bacc(
        "TRN2", target_bir_lowering=False, debug=False, num_devices=N_CORES
    )
    t_table = nc.dram_tensor("table", [V, DD], mybir.dt.float32, kind="ExternalInput")
    t_idx = nc.dram_tensor(
        "idx", [P, NQ * (CHUNK // 16)], mybir.dt.int16, kind="ExternalInput"
    )
    t_trel = nc.dram_tensor(
        "trel", [P, NQ * NJ], mybir.dt.float32, kind="ExternalInput"
    )
    t_iota = nc.dram_tensor("iota", [P, P], mybir.dt.float32, kind="ExternalInput")
    t_out = nc.dram_tensor(
        "out", [W, P, DD], mybir.dt.float32, kind="ExternalOutput"
    )

    with tile.TileContext(nc) as tc, ExitStack() as ctx:
        const = ctx.enter_context(tc.tile_pool(name="const", bufs=1))
        gpool = ctx.enter_context(tc.tile_pool(name="g", bufs=3))
        spool = ctx.enter_context(tc.tile_pool(name="s", bufs=4))
        opool = ctx.enter_context(tc.tile_pool(name="o", bufs=2))
        ppool = ctx.enter_context(tc.tile_pool(name="p", bufs=2, space="PSUM"))

        dma_sem = nc.alloc_semaphore("gather_dma")

        idx_all = const.tile([P, NQ * (CHUNK // 16)], mybir.dt.int16)
        nc.sync.dma_start(idx_all[:], t_idx[:])
        trel_all = const.tile([P, NQ * NJ], mybir.dt.float32)
        nc.sync.dma_start(trel_all[:], t_trel[:])
        iota_t = const.tile([P, P], mybir.dt.float32)
        nc.sync.dma_start(iota_t[:], t_iota[:])

        nc.gpsimd.load_library(mlp)

        gctr = 0
        for w in range(W):
            psum = ppool.tile([P, DD], mybir.dt.float32, space="PSUM")
            for c in range(NCH):
                q = w * NCH + c
                g = gpool.tile([P, NJ, DD], mybir.dt.float32, tag="g")
                nc.gpsimd.dma_gather(
                    g[:],
                    t_table[:],
                    idx_all[:, q * (CHUNK // 16) : (q + 1) * (CHUNK // 16)],
                    CHUNK,
                    CHUNK,
                    DD,
                    # single-packet mode caps num_idxs at 16 engines x 64
                    # descs = 1024; beyond that the packet is malformed and
                    # wedges the device
                    single_packet=False,
                ).then_inc(dma_sem, 16)
                gctr += 1
                for j in range(NJ):
                    t = q * NJ + j
                    s = spool.tile([P, P], mybir.dt.float32, tag="s")
                    nc.vector.tensor_scalar(
                        out=s[:],
                        in0=iota_t[:],
                        scalar1=trel_all[:, t : t + 1],
                        scalar2=None,
                        op0=mybir.AluOpType.is_equal,
                    )
                    mm = nc.tensor.matmul(
                        out=psum[:],
                        lhsT=s[:],
                        rhs=g[:, j, :],
                        start=(c == 0 and j == 0),
                        stop=(c == NCH - 1 and j == NJ - 1),
                    )
                    if j == 0:
                        mm._wait_ge(dma_sem, 16 * gctr)
            ot = opool.tile([P, DD], mybir.dt.float32, tag="o")
            nc.vector.tensor_copy(out=ot[:], in_=psum[:])
            nc.sync.dma_start(t_out[w], ot[:])

    nc.compile()
    return nc


def _pack_inputs(token_ids, tree_ids):
    tok = np.ascontiguousarray(np.asarray(token_ids, dtype=np.int32))
    tree = np.ascontiguousarray(np.asarray(tree_ids, dtype=np.int32))

    bounds = np.searchsorted(tree, np.arange(0, NSEG + 1, SEGS_PW))
    counts = np.diff(bounds)
    assert counts.max() <= CAP, f"window overflow: {counts.max()} > {CAP}"

    tok_pad = np.zeros((WG, CAP), dtype=np.int16)
    trel_pad = np.zeros((WG, CAP), dtype=np.float32)
    for wg in range(WG):
        s, e = bounds[wg], bounds[wg + 1]
        n = e - s
        tok_pad[wg, :n] = tok[s:e].astype(np.int16)
        trel_pad[wg, :n] = (tree[s:e] - SEGS_PW * wg).astype(np.float32)

    # idx: per chunk, index k lives at [16g + k%16, k//16], replicated g=0..7
    idx = (
        tok_pad.reshape(N_CORES, W, NCH, CHUNK // 16, 16)
        .transpose(0, 4, 1, 2, 3)
        .reshape(N_CORES, 16, NQ * (CHUNK // 16))
    )
    idx = np.broadcast_to(idx[:, None, :, :], (N_CORES, 8, 16, NQ * (CHUNK // 16)))
    idx = np.ascontiguousarray(idx.reshape(N_CORES, P, NQ * (CHUNK // 16)))

    # trel: column t = q*NJ + j, row p -> token k = j*128 + p of chunk q
    trel = np.ascontiguousarray(
        trel_pad.reshape(N_CORES, W, NCH, NJ, P)
        .transpose(0, 4, 1, 2, 3)
        .reshape(N_CORES, P, NQ * NJ)
    )
    return idx, trel


def kernel(token_ids, tree_ids, C_hop, C_hop1, batch_size, max_trees):
    global _compiled
    batch_size = int(batch_size)
    max_trees = int(max_trees)
    assert batch_size * max_trees == NSEG

    table = np.ascontiguousarray(
        np.concatenate(
            [np.asarray(C_hop, np.float32), np.asarray(C_hop1, np.float32)], axis=1
        )
    )
    idx, trel = _pack_inputs(token_ids, tree_ids)
    iota = np.ascontiguousarray(
        np.broadcast_to(np.arange(P, dtype=np.float32), (P, P))
    )

    if _compiled is None:
        _compiled = _build_program()
    nc = _compiled

    in_maps = [
        {"table": table, "idx": idx[c], "trel": trel[c], "iota": iota}
        for c in range(N_CORES)
    ]
    res = run_bass_kernel_spmd(nc, in_maps, core_ids=list(range(N_CORES)))

    # assemble: res[c]["out"][w, s, :] = concat row for segment 2048c + 128w + s
    allseg = np.concatenate(
        [res.results[c]["out"].reshape(W * P, DD) for c in range(N_CORES)], axis=0
    )  # [16384, 256]
    key = allseg[:, :D].reshape(batch_size, max_trees, D)
    val = allseg[:, D:].reshape(batch_size, max_trees, D)
    return np.stack([key, val]).astype(np.float32)
